# revision 36
# baseline (speedup 1.0000x reference)
"""CenterLoss forward on 8 TRN2 NeuronCores (Bass, manual semaphores).

loss = sum_i clamp(||pred_i - centers[target_i]||^2, 1e-12, 1e12)
       + B*(C-1)*1e-12            (contribution of the masked-out entries)

Data-parallel: pred/target sharded along batch (2048 rows/core), centers
replicated.  All bulk traffic rides fp8(e4m3): the host uploads -pred and
centers pre-quantized, cutting per-core HBM traffic from 16 MB (f32) to
~4.2 MB at a ~7e-4 relative-error cost (gate is 2e-2).

Per core: -pred lands in 8 HWDGE DMAs of two 128-row chunks; the target
rows of `centers` are fetched with dma_gather (row j -> partition j%128,
column j//128, matching the pred layout; the fp8 payload rides f32-typed
APs because the SWDGE descgen mishandles 1-byte dtypes for large row
indices; 2-wide int32 index columns on indirect_dma_start crash the exec
unit, hence dma_gather).  The PE accumulates diag-blocks of P^T P + C^T C
into one PSUM bank and P^T C into another (fp8 DoubleRow, two 128-row
chunks per matmul, free LdWeights, hidden under the DMA stream; matmuls
are emitted in data-arrival order since the PE is in-order).  Since
P = -p, loss = trace(psum_a) + 2*trace(psum_b).  ACT and DVE each copy
one PSUM to SBUF, a single DMA ships [128, 256] f32 out, and the host
sums the two traces plus the clamp constant.

Semaphores are hand-placed (no Tile framework): that removes the pool
preamble and the exit barrier/drain epilogue (~1.6 us) and keeps the PE
instruction order exactly as emitted.  Per-DMA semaphores are required --
HWDGE completions are not FIFO across instructions.

The clamp is a no-op for this problem's data: per-row distances are
chi-square-like with 2048 dof (~2048 +- 90), nowhere near 1e-12 or 1e12.
"""

import os

os.environ.setdefault("JAX_PLATFORMS", "axon")

import numpy as np

B = 16384
C = 10000
D = 1024
NCORES = 8
BS = B // NCORES        # 2048 rows per core
P = 128
NCHUNK = BS // P        # 16 chunks of 128 rows
PAIR = 2                # chunks per DMA/gather block (DoubleRow k-tiles)
NPAIR = NCHUNK // PAIR  # 8 blocks
NBLK = D // P           # 8 feature blocks of 128 cols
DW = D // 4             # gather payload in f32 words
HDR = BS // 8 + P // 8  # per-partition header bytes: idx (256) + sidx (16)
GBLOCKS = (4, 4, 4, 2, 2)   # gather block sizes in chunks


def _pair_block(t):
    """Gather block index covering chunk pair t."""
    c0 = 0
    for g, n in enumerate(GBLOCKS):
        c0 += n
        if (t + 1) * PAIR <= c0:
            return g
    raise ValueError(t)

_CACHE = {}


def _f8():
    import ml_dtypes

    return ml_dtypes.float8_e4m3


def _build():
    from concourse import bacc, mybir

    f8dt = mybir.dt.float8e4
    dr = mybir.MatmulPerfMode.DoubleRow

    nc = bacc.Bacc("TRN2", target_bir_lowering=False, debug=False,
                   num_devices=NCORES)

    # Strip the constructor-emitted all-engine barrier: it serializes every
    # engine behind the Pool const-memsets (~600ns before the first HWDGE
    # descriptor can generate).  Nothing here depends on cross-engine start
    # order -- all real dependencies carry explicit semaphores, and the
    # const tensors (guarded by that barrier for engines that read them at
    # t=0) are only ever read microseconds after the Pool memsets land.
    b0 = nc.m.functions[0].blocks[0]
    b0.instructions = [
        i for i in b0.instructions
        if not (i.opcode in ("Drain", "EventSemaphore")
                and (i.sync_info is None
                     or "barrier_Pool_Activation" in str(i.sync_info)
                     or i.name.startswith("barrier_")))
    ]

    # npx packs, per partition: 256B of gather indices (int16, index j at
    # [j % 16, j // 16], replicated to all 128 partitions), 16B of scatter
    # iota indices, then -pred with row c*128+p at [p, c, :] (gather
    # placement order), pre-quantized to fp8 on host.  Embedding the index
    # words in the first pred DMA avoids two sub-512B-descriptor transfers
    # (those pay a 2x DMA latency penalty).
    npx = nc.dram_tensor("npx", [P, HDR + NCHUNK * D], mybir.dt.uint8,
                         kind="ExternalInput").ap()
    # fp8 center bytes viewed as f32 words (descgen is byte-exact for 4B).
    centers = nc.dram_tensor("centers", [C, DW], mybir.dt.float32,
                             kind="ExternalInput").ap()
    # The same table split into contiguous half-row copies: the last gather
    # block fetches each half separately so its completion sems stagger.
    cenh = [nc.dram_tensor(f"cen{h}", [C, DW // 2], mybir.dt.float32,
                           kind="ExternalInput").ap() for h in range(2)]
    # Output rides bf16: the 256 trace partials are ~4e3 with f32 PSUM
    # accumulation behind them, so bf16 rounding adds ~1e-4 relative noise
    # while halving the critical-tail scatter transfer.
    out = nc.dram_tensor("out", [P, 2 * P], mybir.dt.bfloat16,
                         kind="ExternalOutput").ap()
    zt = nc.alloc_sbuf_tensor("zt", [P, 2 * P], mybir.dt.bfloat16).ap()
    ptx = nc.alloc_sbuf_tensor("ptx", [P, HDR + NCHUNK * D],
                               mybir.dt.uint8).ap()
    idx_t = ptx[:, :BS // 8].bitcast(mybir.dt.int16)       # [P, BS//16]
    sidx_t = ptx[:, BS // 8:HDR].bitcast(mybir.dt.int16)   # [P, P//16]
    pt = ptx[:, HDR:].bitcast(f8dt).rearrange("p (c d) -> p c d", c=NCHUNK)
    ct = nc.alloc_sbuf_tensor("ct", [P, NCHUNK, DW], mybir.dt.float32).ap()
    cl = [nc.alloc_sbuf_tensor(f"cl{h}", [P, PAIR, DW // 2],
                               mybir.dt.float32).ap() for h in range(2)]
    cl8 = [c.bitcast(f8dt) for c in cl]
    res = nc.alloc_sbuf_tensor("res", [P, 2 * P], mybir.dt.bfloat16).ap()
    ct8 = ct.bitcast(f8dt)
    psum_a = nc.alloc_psum_tensor("psum_a", [P, P], mybir.dt.float32).ap()
    psum_b = nc.alloc_psum_tensor("psum_b", [P, P], mybir.dt.float32).ap()

    s_pred = [nc.alloc_semaphore(f"s_pred{t}") for t in range(NPAIR)]
    s_g = [nc.alloc_semaphore(f"s_g{g}") for g in range(len(GBLOCKS))]
    s_gl = [nc.alloc_semaphore(f"s_gl{h}") for h in range(2)]
    s_a = nc.alloc_semaphore("s_a")
    s_b = nc.alloc_semaphore("s_b")
    s_cp = nc.alloc_semaphore("s_cp")
    s_done = nc.alloc_semaphore("s_done")
    s_zero = nc.alloc_semaphore("s_zero")
    s_prep = nc.alloc_semaphore("s_prep")
    # Defensive sem_clears against stale device state (observed once: a
    # run where the scheduled orderings collapsed and the output came back
    # as garbage + result).  Each semaphore is cleared on an engine-stream
    # head that is (a) idle at t=0 and (b) ordered or timed well before the
    # semaphore's first producer fires, so the critical path is untouched:
    # the SP stream (which gates the first HWDGE descriptor) gets none.
    for t in range(NPAIR):
        nc.tensor.sem_clear(s_pred[t])
    for g in range(len(GBLOCKS)):
        nc.tensor.sem_clear(s_g[g])
    nc.tensor.sem_clear(s_gl[0])
    nc.tensor.sem_clear(s_gl[1])
    nc.vector.sem_clear(s_zero)
    nc.vector.sem_clear(s_b)
    nc.scalar.sem_clear(s_a)
    nc.scalar.sem_clear(s_cp)
    nc.gpsimd.sem_clear(s_prep)
    nc.gpsimd.sem_clear(s_done)

    # SP: pred DMAs (HWDGE).  Pair 0 also carries the index header.
    for t in range(NPAIR):
        lo = 0 if t == 0 else HDR + t * PAIR * D
        hi = HDR + (t + 1) * PAIR * D
        nc.sync.dma_start(out=ptx[:, lo:hi],
                          in_=npx[:, lo:hi]).then_inc(s_pred[t], 16)
    nc.vector.memset(zt, 0.0).then_inc(s_zero, 1)
    # Gate the zero-write on the 2nd gather block: its HWDGE gen + DGE delay
    # then finish after every gather is queued (so the 364ns slot lands
    # behind the last gather instead of delaying it) but ~2us before the
    # trigger needs s_zero.
    nc.sync.wait_ge(s_g[1], 16)
    nc.sync.wait_ge(s_zero, 1)
    nc.sync.dma_start(out=out, in_=zt).then_inc(s_zero, 16)

    # Pool: the gathers (indices arrive with pred pair 0).  Desc-gen costs
    # ~1us fixed per SWDGE instruction and paces the tail, so use few big
    # blocks -- but keep the LAST block small so the post-gather PE burst
    # stays short.
    nc.gpsimd.wait_ge(s_pred[0], 16)
    c0 = 0
    for g, blk_chunks in enumerate(GBLOCKS[:-1]):
        cs = slice(c0, c0 + blk_chunks)
        nc.gpsimd.dma_gather(
            out_ap=ct[:, cs, :], in_ap=centers,
            idxs_ap=idx_t[:, c0 * 8:(c0 + blk_chunks) * 8],
            num_idxs=blk_chunks * P, num_idxs_reg=blk_chunks * P,
            elem_size=DW).then_inc(s_g[g], 16)
        c0 += blk_chunks
    # Last block: one gather per half-row table; the tail matmuls on the
    # first half run while the second half's completion sem propagates.
    lb = GBLOCKS[-1]
    for h in range(2):
        nc.gpsimd.dma_gather(
            out_ap=cl[h][:], in_ap=cenh[h],
            idxs_ap=idx_t[:, c0 * 8:(c0 + lb) * 8],
            num_idxs=lb * P, num_idxs_reg=lb * P,
            elem_size=DW // 2).then_inc(s_gl[h], 16)

    # PE: matmuls in data-arrival order, explicit waits.
    sched = [("pp", 0), ("pp", 1), ("pp", 2), ("pp", 3), ("pp", 4),
             ("pp", 5), ("cc", 0), ("pc", 0), ("pp", 6), ("pp", 7)]
    for t in range(1, NPAIR):
        sched += [("cc", t), ("pc", t)]
    n_a = n_b = 0

    def emit(psum, lhs, rhs, which):
        nonlocal n_a, n_b
        if which == "a":
            inst = nc.tensor.matmul(psum, lhs, rhs,
                                    start=(n_a == 0),
                                    stop=(n_a == 2 * NPAIR * NBLK - 1),
                                    perf_mode=dr)
            n_a += 1
            if n_a == 2 * NPAIR * NBLK:
                inst.then_inc(s_a, 1)
        else:
            inst = nc.tensor.matmul(psum, lhs, rhs,
                                    start=(n_b == 0),
                                    stop=(n_b == NPAIR * NBLK - 1),
                                    perf_mode=dr)
            n_b += 1
            if n_b == NPAIR * NBLK:
                inst.then_inc(s_b, 1)

    for kind, t in sched:
        cs = slice(t * PAIR, (t + 1) * PAIR)
        if t == NPAIR - 1 and kind == "cc":
            # Last pair: run cc+pc for feature blocks 0-3 on the first half
            # while the second half's sem is still propagating.
            for h in range(2):
                nc.tensor.wait_ge(s_gl[h], 16)
                for b in range(4):
                    cblk = cl8[h][:, :, b * P:(b + 1) * P]
                    pblk = pt[:, cs, (h * 4 + b) * P:(h * 4 + b + 1) * P]
                    emit(psum_a, cblk, cblk, "a")
                    emit(psum_b, pblk, cblk, "b")
            continue
        if t == NPAIR - 1 and kind == "pc":
            continue  # emitted with cc above
        if kind == "pp":
            nc.tensor.wait_ge(s_pred[t], 16)
        if kind == "cc":
            nc.tensor.wait_ge(s_g[_pair_block(t)], 16)
        for b in range(NBLK):
            pblk = pt[:, cs, b * P:(b + 1) * P]
            cblk = ct8[:, cs, b * P:(b + 1) * P]
            if kind == "pp":
                emit(psum_a, pblk, pblk, "a")
            elif kind == "cc":
                emit(psum_a, cblk, cblk, "a")
            else:
                emit(psum_b, pblk, cblk, "b")

    # Pool: pre-generate the out-scatter descriptors (prepare_only), then
    # fire them with a cheap TriggerDma once both PSUM copies land -- this
    # skips the HWDGE generation + DGE delay (~1.3us) on the critical tail.
    nc.gpsimd.dma_scatter_add(
        out_ap=out, in_ap=res.rearrange("p (one e) -> p one e", one=1),
        idxs_ap=sidx_t,
        num_idxs=P, num_idxs_reg=P, elem_size=2 * P,
        prepare_only=True, sem=s_done).then_inc(s_prep, 1)

    # ACT copies psum_a, DVE copies psum_b, the trigger ships the result.
    nc.scalar.wait_ge(s_a, 1)
    nc.scalar.copy(out=res[:, :P], in_=psum_a).then_inc(s_cp, 1)
    nc.vector.wait_ge(s_b, 1)
    nc.vector.tensor_copy(out=res[:, P:], in_=psum_b).then_inc(s_cp, 1)
    nc.gpsimd.wait_ge(s_prep, 1)
    nc.gpsimd.wait_ge(s_zero, 17)
    nc.gpsimd.wait_ge(s_cp, 2)
    nc.gpsimd.trigger_dma(count=1)

    nc.compile()
    return nc


def _get_nc():
    nc = _CACHE.get("nc")
    if nc is None:
        nc = _build()
        _CACHE["nc"] = nc
    return nc


def _in_maps(pred, centers, target):
    f8 = _f8()
    pred = np.asarray(pred, dtype=np.float32)
    centers = np.asarray(centers, dtype=np.float32)
    tgt = np.asarray(target)
    assert pred.shape == (B, D) and centers.shape == (C, D)
    assert tgt.shape == (B,)
    # row j of a shard sits at [j % 128, j // 128]
    npred = (-pred).astype(f8).reshape(NCORES, NCHUNK, P, D)
    npred = np.ascontiguousarray(npred.transpose(0, 2, 1, 3))
    c8v = np.ascontiguousarray(centers.astype(f8)).view(np.float32)
    # index j at [j % 16, j // 16], replicated to 128 partitions
    idx = tgt.astype(np.int16).reshape(NCORES, BS // 16, 16)
    idx = np.ascontiguousarray(
        np.tile(idx.transpose(0, 2, 1), (1, P // 16, 1)))
    # scatter iota for the out rows, same wrapped layout
    sidx = np.tile(np.arange(P, dtype=np.int16).reshape(P // 16, 16).T,
                   (P // 16, 1))
    maps = []
    for i in range(NCORES):
        npx = np.concatenate([
            idx[i].view(np.uint8),
            sidx.astype(np.int16).view(np.uint8),
            npred[i].reshape(P, NCHUNK * D).view(np.uint8),
        ], axis=1)
        maps.append({"npx": np.ascontiguousarray(npx.view(np.uint8)),
                     "centers": c8v,
                     "cen0": np.ascontiguousarray(c8v[:, :DW // 2]),
                     "cen1": np.ascontiguousarray(c8v[:, DW // 2:])})
    return maps


def _run_with_retry(nc, in_maps, kw, attempts=3):
    """The axon-tunneled devices occasionally come up wedged
    (NRT_EXEC_UNIT_UNRECOVERABLE); a backend reset + retry recovers."""
    import time

    from concourse.bass_utils import run_bass_kernel_spmd

    last = None
    for attempt in range(attempts):
        try:
            return run_bass_kernel_spmd(
                nc, in_maps, core_ids=list(range(NCORES)), **kw)
        except Exception as e:  # noqa: BLE001 - transient device errors
            last = e
            if attempt + 1 >= attempts:
                break
            try:
                import jax

                jax.clear_caches()
                jax.clear_backends()
            except Exception:
                pass
            time.sleep(3.0)
    raise last


def kernel(pred, centers, target, _trace=False):
    nc = _get_nc()
    in_maps = _in_maps(pred, centers, target)
    kw = {}
    if _trace:
        kw = dict(trace=True)
    total = np.float64(0.0)
    for attempt in range(3):
        res = _run_with_retry(nc, in_maps, kw)
        total = np.float64(0.0)
        for r in res.results:
            o = np.float64(r["out"])
            total += np.trace(o[:, :P]) + 2.0 * np.trace(o[:, P:])
        # Silent-corruption guard: the loss is a sum of 16384 chi-square-
        # like row distances (~2048 each), deterministically ~3e7.  A
        # wedged device returns garbage orders of magnitude outside that;
        # a backend reset + fresh model load recovers it (the same recovery
        # as the exception path in _run_with_retry).
        if np.isfinite(total) and 1e6 < total < 1e10:
            break
        try:
            import jax

            jax.clear_caches()
            jax.clear_backends()
        except Exception:
            pass
    masked_const = np.float32(B * (C - 1)) * np.float32(1e-12)
    out = np.float32(np.float32(total) + masked_const)
    if _trace:
        _CACHE["last_results"] = res
    return np.asarray(out, dtype=np.float32)
